# revision 1
# baseline (speedup 1.0000x reference)
"""AttentionBlock (B=4, C=256, H=W=64) on 8 Trainium2 NeuronCores.

Sharding: data-parallel over (batch, query-half): core i handles batch i//2,
query pixels [half*2048, (half+1)*2048), half = i%2. GroupNorm stats + k/vT
are computed per batch element (duplicated across the pair, cheap); the
O(N^2) attention work is fully sharded 8 ways. No collectives.

Device algorithm per core (pixels m,n in [0,4096), channels c in [0,256)):
  1. GroupNorm stats via bn_stats/bn_aggr per channel + tiny matmuls with 0/1
     group matrices to sum/broadcast across partitions;
     rstd = 1/sqrt(var+eps) via ACT Sqrt + DVE reciprocal.
  2. The normalize step is folded into the convolution weights on device:
     qkv(gn(x)) = (W .* scale_c) x + (W bias_c + b). So the qkv matmuls run
     directly on a f32r copy of x (made during the DMA head, off the stats
     critical path).
  3. k = Wk' x, q = Wq' xh, vT[m,c] = x_m^T Wv'^T (v produced pre-transposed
     so the attention O-matmul needs no transposes).
  4. Attention per 512-wide query chunk, looping 32 key blocks mb:
     S^T[mb,n] (PSUM) <- k_mb^T q;  E = exp(S/16) (ACT PSUM->SBUF, f32r);
     O[c,n] += vT_mb^T E (PSUM accum);  R[n] += ones^T E (M=1 PSUM accum).
     proj runs directly on O (proj is linear, so /R commutes past it);
     Rinv = DVE reciprocal, partition-broadcast by a K=1 ones matmul;
     out = proj(O)*Rinv + xh' (xh' = x_half + all foldable biases, host-made).
All big matmuls run in float32r (TF32-like, ~1.6e-4 rel err, full PE rate).
"""

import numpy as np

B, C, HW = 4, 256, 4096
NH = 2048            # query pixels per core
G, CPG = 32, 8       # groups, channels per group
EPS = 1e-5
MB = HW // 128       # 32 key blocks

_cache = {}


def build_nc():
    """Build (and cache) the Bass module."""
    if "nc" in _cache:
        return _cache["nc"]
    import concourse.tile as tile
    from concourse import bacc, mybir

    f32 = mybir.dt.float32
    f32r = mybir.dt.float32r
    AF = mybir.ActivationFunctionType
    OP = mybir.AluOpType

    nc = bacc.Bacc("TRN2", target_bir_lowering=False, debug=False,
                   enable_asserts=False, num_devices=8)

    # ---- DRAM I/O (host preps everything into device layout) ----
    d_xf = nc.dram_tensor("xf", [128, 2, HW], f32r, kind="ExternalInput")
    d_xh = nc.dram_tensor("xh", [128, 2, NH], f32, kind="ExternalInput")
    d_wq = nc.dram_tensor("wq", [128, 2, C], f32r, kind="ExternalInput")
    d_wk = nc.dram_tensor("wk", [128, 2, C], f32r, kind="ExternalInput")
    d_wv = nc.dram_tensor("wv", [128, 2, C], f32r, kind="ExternalInput")
    d_wp = nc.dram_tensor("wp", [128, 2, C], f32r, kind="ExternalInput")
    d_sb = nc.dram_tensor("sb", [128, 2, 5], f32, kind="ExternalInput")
    d_ag = nc.dram_tensor("ag", [128, 2, G], f32, kind="ExternalInput")
    d_bg = nc.dram_tensor("bg", [G, 2, 128], f32, kind="ExternalInput")
    d_out = nc.dram_tensor("out", [128, 2, NH], f32, kind="ExternalOutput")

    with tile.TileContext(nc) as tc:
        with (
            tc.tile_pool(name="big", bufs=1) as big,
            tc.tile_pool(name="cst", bufs=1) as cst,
            tc.tile_pool(name="wrk", bufs=2) as wrk,
            tc.tile_pool(name="epool", bufs=5) as epool,
            tc.tile_pool(name="gnp", bufs=1) as gnp,
            tc.tile_pool(name="ps_s", bufs=4, space="PSUM") as ps_s,
            tc.tile_pool(name="ps_o", bufs=1, space="PSUM") as ps_o,
            tc.tile_pool(name="ps_t", bufs=1, space="PSUM") as ps_t,
        ):
            # ---- weight/constant loads first (scalar queue; x loads below
            # saturate the sync queue) ----
            # pre-warm ACT table sets before any other ACT work: exp first,
            # sqrt second, so the resident set is sqrt when GN needs it; the
            # attention exp then reloads once, hidden behind qkv matmuls.
            warm = cst.tile([1, 2], f32, tag="warm")
            nc.vector.memset(warm, 1.0)
            nc.scalar.activation(out=warm[:, 0:1], in_=warm[:, 0:1],
                                 func=AF.Exp)
            nc.scalar.activation(out=warm[:, 1:2], in_=warm[:, 1:2],
                                 func=AF.Sqrt)
            smalls = cst.tile([128, 2, 5], f32, tag="smalls")
            nc.scalar.dma_start(out=smalls, in_=d_sb.ap())
            qb = smalls[:, :, 0:1]
            kb = smalls[:, :, 1:2]
            gb = smalls[:, :, 3:4]
            rbias = smalls[:, :, 4:5]
            ag = cst.tile([128, 2, G], f32, tag="ag")
            nc.scalar.dma_start(out=ag, in_=d_ag.ap())
            bg = cst.tile([G, 2, 128], f32, tag="bg")
            nc.scalar.dma_start(out=bg, in_=d_bg.ap())

            # ---- input loads (f32r straight from DRAM; HW rounds on read)
            xfr = big.tile([128, 2, HW], f32r, tag="xfr")
            for ci in range(2):
                for j in range(8):
                    sl = slice(j * 512, (j + 1) * 512)
                    eng = nc.sync if (j % 2 == 0) else nc.scalar
                    eng.dma_start(out=xfr[:, ci, sl], in_=d_xf.ap()[:, ci, sl])
            wall = cst.tile([128, 2, 4 * C], f32r, tag="wall")
            for i, d in enumerate((d_wq, d_wk, d_wv, d_wp)):
                nc.scalar.dma_start(out=wall[:, :, i * C:(i + 1) * C], in_=d.ap())
            xh = big.tile([128, 2, NH], f32, tag="xh")
            xhr = big.tile([128, 2, NH], f32r, tag="xhr")
            xo = big.tile([128, 2, NH], f32, tag="xo")  # x_half + rbias
            for ci in range(2):
                for j in range(2):
                    sl = slice(j * 1024, (j + 1) * 1024)
                    nc.sync.dma_start(out=xh[:, ci, sl], in_=d_xh.ap()[:, ci, sl])
                    nc.vector.tensor_copy(out=xhr[:, ci, sl], in_=xh[:, ci, sl])
                    nc.vector.tensor_scalar(
                        out=xo[:, ci, sl], in0=xh[:, ci, sl],
                        scalar1=rbias[:, ci, :], scalar2=None, op0=OP.add)

            onesc = cst.tile([128, 2], f32, tag="onesc")
            nc.vector.memset(onesc, 1.0)
            epst = cst.tile([G, 1], f32, tag="epst")
            nc.vector.memset(epst, EPS)
            ones_col = cst.tile([128, 1], f32r, tag="ones_col")  # R lhsT
            nc.vector.tensor_copy(out=ones_col, in_=onesc[:, 0:1])
            onesr = cst.tile([1, 128], f32, tag="onesr")
            nc.vector.memset(onesr, 1.0)
            ones_row = cst.tile([1, 128], f32r, tag="ones_row")  # bcast lhsT
            nc.vector.tensor_copy(out=ones_row, in_=onesr)


            # ---- GroupNorm stats ----
            bstat = gnp.tile([128, 2, 8, 6], f32, tag="bstat")
            for ci in range(2):
                for j in range(8):
                    nc.vector.bn_stats(
                        out=bstat[:, ci, j, :],
                        in_=xfr[:, ci, j * 512:(j + 1) * 512])
            stats2 = gnp.tile([128, 2, 2], f32, tag="stats2")  # (mean, E[x^2])
            tmp1 = gnp.tile([128, 1], f32, tag="tmp1")
            for ci in range(2):
                nc.vector.bn_aggr(out=stats2[:, ci, :], in_=bstat[:, ci, :, :])
                nc.vector.tensor_tensor(
                    out=tmp1, in0=stats2[:, ci, 0:1], in1=stats2[:, ci, 0:1],
                    op=OP.mult)
                nc.vector.tensor_tensor(
                    out=stats2[:, ci, 1:2], in0=stats2[:, ci, 1:2], in1=tmp1,
                    op=OP.add)
            # group sums across partitions: [G, 2] = sum_ci ag[ci]^T stats2[ci]
            pg = ps_t.tile([G, 2], f32, tag="t")
            for ci in range(2):
                nc.tensor.matmul(pg, lhsT=ag[:, ci, :], rhs=stats2[:, ci, :],
                                 start=(ci == 0), stop=(ci == 1))
            # ag carries 1/CPG so pg is directly (mean_g, E[x^2]_g)
            pgs = gnp.tile([G, 2], f32, tag="pgs")
            nc.vector.tensor_copy(out=pgs, in_=pg)
            gst = gnp.tile([G, 4], f32, tag="gst")  # mean^2, var, sd, -
            nc.vector.tensor_tensor(out=gst[:, 0:1], in0=pgs[:, 0:1],
                                    in1=pgs[:, 0:1], op=OP.mult)
            nc.vector.tensor_tensor(out=gst[:, 1:2], in0=pgs[:, 1:2],
                                    in1=gst[:, 0:1], op=OP.subtract)
            gfin = gnp.tile([G, 2], f32, tag="gfin")  # (rstd_g, mean_g*rstd_g)
            nc.scalar.activation(out=gst[:, 2:3], in_=gst[:, 1:2],
                                 func=AF.Sqrt, bias=epst)
            nc.vector.reciprocal(out=gfin[:, 0:1], in_=gst[:, 2:3])
            nc.vector.tensor_tensor(out=gfin[:, 1:2], in0=pgs[:, 0:1],
                                    in1=gfin[:, 0:1], op=OP.mult)
            # bg carries gn_w, so pbc = (scale_c, mean_c*scale_c);
            # bias_c = gn_b - mean_c*scale_c
            scbc = gnp.tile([128, 2, 2], f32, tag="scbc")
            for ci in range(2):
                pbc = ps_t.tile([128, 2], f32, tag="t")
                nc.tensor.matmul(pbc, lhsT=bg[:, ci, :], rhs=gfin,
                                 start=True, stop=True)
                nc.vector.tensor_copy(out=scbc[:, ci, 0:1], in_=pbc[:, 0:1])
                nc.vector.tensor_tensor(out=scbc[:, ci, 1:2], in0=gb[:, ci, :],
                                        in1=pbc[:, 1:2], op=OP.subtract)

            # ---- fold GN into conv weights ----
            # 1) unscaled f32r copy; 2) bias matmuls on unscaled W;
            # 3) scale q/k/v weights in place: W' = W .* scale_c (per c_in)
            wqs = wall[:, :, 0 * C:1 * C]
            wks = wall[:, :, 1 * C:2 * C]
            wvs = wall[:, :, 2 * C:3 * C]
            wp = wall[:, :, 3 * C:4 * C]
            # scale weights in place FIRST (gates the qkv matmuls);
            # the bias chain below then uses scaled W with b/s instead
            for ci in range(2):
                nc.vector.tensor_scalar(
                    out=wall[:, ci, 0:3 * C], in0=wall[:, ci, 0:3 * C],
                    scalar1=scbc[:, ci, 0:1], scalar2=None, op0=OP.mult)
            # bsr = bias_c / scale_c (so W' @ bsr == W @ bias_c); 2 copies:
            # f32r matmuls need an even moving free dim
            sinv = gnp.tile([128, 2, 1], f32, tag="sinv")
            bsr = gnp.tile([128, 2, 1], f32, tag="bsr")
            bcr = cst.tile([128, 2, 2], f32r, tag="bcr")
            for ci in range(2):
                nc.vector.reciprocal(out=sinv[:, ci, :], in_=scbc[:, ci, 0:1])
                nc.vector.tensor_tensor(out=bsr[:, ci, :], in0=scbc[:, ci, 1:2],
                                        in1=sinv[:, ci, :], op=OP.mult)
                nc.vector.tensor_copy(out=bcr[:, ci, 0:1], in_=bsr[:, ci, :])
                nc.vector.tensor_copy(out=bcr[:, ci, 1:2], in_=bsr[:, ci, :])
            # effective channel biases: qb2/kb2 = b + W bias_c (unscaled W)
            bias2 = gnp.tile([128, 2, 2], f32, tag="bias2")  # cols: qb2, kb2
            for wi, wsl in enumerate((wqs, wks)):
                for cb in range(2):
                    pbias = ps_t.tile([128, 2], f32, tag="t")
                    for ci in range(2):
                        nc.tensor.matmul(
                            pbias,
                            lhsT=wsl[:, ci, cb * 128:(cb + 1) * 128],
                            rhs=bcr[:, ci, :], start=(ci == 0), stop=(ci == 1))
                    nc.vector.tensor_tensor(
                        out=bias2[:, cb, wi:wi + 1], in0=pbias[:, 0:1],
                        in1=(qb if wi == 0 else kb)[:, cb, :], op=OP.add)
            # v bias along FREE dim: vb2[1, c_out] = bias_c^T Wv (unscaled)
            pvb = ps_t.tile([1, 512], f32, tag="t")
            for ci in range(2):
                nc.tensor.matmul(pvb[:, 0:C], lhsT=bcr[:, ci, 0:1],
                                 rhs=wvs[:, ci, :], start=(ci == 0),
                                 stop=(ci == 1))
            vb2r = gnp.tile([1, C], f32r, tag="vb2r")
            nc.scalar.copy(out=vb2r, in_=pvb[:, 0:C])
            vb2b = gnp.tile([128, C], f32, tag="vb2b")
            pvbb = ps_t.tile([128, 512], f32, tag="t")
            nc.tensor.matmul(pvbb[:, 0:C], lhsT=ones_row, rhs=vb2r,
                             start=True, stop=True)
            nc.scalar.copy(out=vb2b, in_=pvbb[:, 0:C])

            nc.scalar.activation(out=warm[:, 0:1], in_=warm[:, 0:1],
                                 func=AF.Exp)
            # ---- qkv matmuls (on x directly; weights carry the GN fold) ----
            kt = big.tile([128, 2, HW], f32r, tag="xfr2")
            for cb in range(2):
                for j in range(8):
                    sl = slice(j * 512, (j + 1) * 512)
                    pk = ps_s.tile([128, 512], f32, tag="s")
                    for ci in range(2):
                        nc.tensor.matmul(
                            pk, lhsT=wks[:, ci, cb * 128:(cb + 1) * 128],
                            rhs=xfr[:, ci, sl], start=(ci == 0), stop=(ci == 1))
                    nc.scalar.activation(out=kt[:, cb, sl], in_=pk,
                                         func=AF.Identity,
                                         bias=bias2[:, cb, 1:2])
            vT = big.tile([128, MB, C], f32r, tag="vT")
            for mb in range(MB):
                msl = slice(mb * 128, (mb + 1) * 128)
                pv = ps_s.tile([128, 512], f32, tag="s")
                for ci in range(2):
                    nc.tensor.matmul(pv[:, 0:C], lhsT=xfr[:, ci, msl],
                                     rhs=wvs[:, ci, :],
                                     start=(ci == 0), stop=(ci == 1))
                nc.vector.tensor_tensor(out=vT[:, mb, :], in0=pv[:, 0:C],
                                        in1=vb2b, op=OP.add)
            qt = big.tile([128, 2, NH], f32r, tag="qt")
            for cb in range(2):
                for j in range(4):
                    sl = slice(j * 512, (j + 1) * 512)
                    pq = ps_s.tile([128, 512], f32, tag="s")
                    for ci in range(2):
                        nc.tensor.matmul(
                            pq, lhsT=wqs[:, ci, cb * 128:(cb + 1) * 128],
                            rhs=xhr[:, ci, sl], start=(ci == 0), stop=(ci == 1))
                    nc.scalar.activation(out=qt[:, cb, sl], in_=pq,
                                         func=AF.Identity,
                                         bias=bias2[:, cb, 0:1])

            # ---- attention ----
            for j in range(NH // 512):
                sl = slice(j * 512, (j + 1) * 512)
                po = ps_o.tile([128, 3, 512], f32, tag="o")  # O c0, O c1, R
                for mb in range(MB):
                    ps = ps_s.tile([128, 512], f32, tag="s")
                    for ci in range(2):
                        nc.tensor.matmul(
                            ps, lhsT=kt[:, ci, mb * 128:(mb + 1) * 128],
                            rhs=qt[:, ci, sl], start=(ci == 0), stop=(ci == 1))
                    et = epool.tile([128, 512], f32r, tag="et")
                    nc.scalar.activation(out=et, in_=ps, func=AF.Exp,
                                         scale=1.0 / 16.0)
                    for cb in range(2):
                        nc.tensor.matmul(
                            po[:, cb, :],
                            lhsT=vT[:, mb, cb * 128:(cb + 1) * 128],
                            rhs=et, start=(mb == 0), stop=(mb == MB - 1),
                            skip_group_check=True)
                    nc.tensor.matmul(
                        po[0:1, 2, :], lhsT=ones_col, rhs=et,
                        start=(mb == 0), stop=(mb == MB - 1),
                        skip_group_check=True)
                # Free po fast: copy R and both O banks out immediately
                # (ACT + DVE in parallel); the slow single-lane reciprocal
                # then runs on the SBUF copy without holding po.
                rsb = wrk.tile([1, 512], f32, tag="rsb")
                nc.vector.tensor_copy(out=rsb, in_=po[0:1, 2, :])
                onorm = wrk.tile([128, 2, 512], f32r, tag="onorm")
                nc.vector.tensor_copy(out=onorm[:, 0, :], in_=po[:, 0, :])
                nc.vector.tensor_copy(out=onorm[:, 1, :], in_=po[:, 1, :])
                rinv = wrk.tile([1, 512], f32r, tag="rinv")
                with nc.allow_low_precision(reason="f32r is full fp32 storage"):
                    nc.vector.reciprocal(out=rinv, in_=rsb)
                last = (j == NH // 512 - 1)
                if last:
                    # final chunk: nothing left to overlap with, so keep PE's
                    # last matmuls off the slow reciprocal chain -- proj runs
                    # first (into ps_t + po's freed O bank), broadcast goes to
                    # po's freed R bank.
                    pps = []
                    for cb in range(2):
                        if cb == 0:
                            pp = ps_t.tile([128, 512], f32, tag="t",
                                           name="pp_last")
                        else:
                            pp = po[:, 1, :]
                        for ci in range(2):
                            nc.tensor.matmul(
                                pp, lhsT=wp[:, ci, cb * 128:(cb + 1) * 128],
                                rhs=onorm[:, ci, :], start=(ci == 0),
                                stop=(ci == 1), skip_group_check=True)
                        pps.append(pp)
                    nc.tensor.matmul(po[:, 2, :], lhsT=ones_row, rhs=rinv,
                                     start=True, stop=True,
                                     skip_group_check=True)
                    rb = wrk.tile([128, 512], f32, tag="rb")
                    nc.vector.tensor_copy(out=rb, in_=po[:, 2, :])
                    for cb in range(2):
                        outt = wrk.tile([128, 512], f32, tag="outt")
                        nc.vector.tensor_tensor(out=outt, in0=pps[cb], in1=rb,
                                                op=OP.mult)
                        nc.vector.tensor_tensor(out=outt, in0=outt,
                                                in1=xo[:, cb, sl], op=OP.add)
                        nc.sync.dma_start(out=d_out.ap()[:, cb, sl], in_=outt)
                else:
                    pbx = ps_t.tile([128, 512], f32, tag="t")
                    nc.tensor.matmul(pbx, lhsT=ones_row, rhs=rinv,
                                     start=True, stop=True)
                    rb = wrk.tile([128, 512], f32, tag="rb")
                    nc.scalar.copy(out=rb, in_=pbx)
                    for cb in range(2):
                        pp = ps_t.tile([128, 512], f32, tag="t")
                        for ci in range(2):
                            nc.tensor.matmul(
                                pp, lhsT=wp[:, ci, cb * 128:(cb + 1) * 128],
                                rhs=onorm[:, ci, :], start=(ci == 0),
                                stop=(ci == 1))
                        outt = wrk.tile([128, 512], f32, tag="outt")
                        nc.vector.tensor_tensor(out=outt, in0=pp, in1=rb,
                                                op=OP.mult)
                        nc.vector.tensor_tensor(out=outt, in0=outt,
                                                in1=xo[:, cb, sl], op=OP.add)
                        nc.sync.dma_start(out=d_out.ap()[:, cb, sl], in_=outt)

    nc.compile()
    _cache["nc"] = nc
    return nc


def _prep_maps(x, gn_w, gn_b, qkv_w, qkv_b, proj_w, proj_b):
    """Host-side sharding + layout prep. Returns list of 8 in_maps."""
    x = np.asarray(x, np.float32)
    qkv_w = np.asarray(qkv_w, np.float32)
    qkv_b = np.asarray(qkv_b, np.float32)
    proj_w = np.asarray(proj_w, np.float32)
    proj_b = np.asarray(proj_b, np.float32)
    gn_w = np.asarray(gn_w, np.float32)
    gn_b = np.asarray(gn_b, np.float32)

    def chunked(a):  # [256, ...] -> [128, 2, ...]
        return np.ascontiguousarray(a.reshape(2, 128, *a.shape[1:]).transpose(
            1, 0, *range(2, a.ndim + 1)))

    wq = chunked(qkv_w[0:C].T.copy())          # [c_in, c_out] -> [128,2,C]
    wk = chunked(qkv_w[C:2 * C].T.copy())
    wv = chunked(qkv_w[2 * C:3 * C].T.copy())
    wp = chunked(proj_w.T.copy())
    rbias = proj_w @ qkv_b[2 * C:3 * C] + proj_b   # v-bias fold + proj bias
    smalls = np.stack([qkv_b[0:C], qkv_b[C:2 * C], gn_w, gn_b, rbias], axis=1)
    smalls = chunked(smalls)

    cidx = np.arange(C)
    ag_full = (cidx[:, None] // CPG == np.arange(G)[None, :]).astype(np.float32)
    ag = chunked(ag_full / CPG)                     # [128, 2, G], carries 1/8
    bg_full = ag_full * gn_w[:, None]               # carries gn_w
    bg = np.ascontiguousarray(
        bg_full.reshape(2, 128, G).transpose(2, 0, 1))  # [G, 2, 128]

    maps = []
    for core in range(8):
        b, half = core // 2, core % 2
        xf = x[b].reshape(C, HW)
        xh = xf[:, half * NH:(half + 1) * NH]
        maps.append({
            "xf": chunked(xf), "xh": chunked(xh),
            "wq": wq, "wk": wk, "wv": wv, "wp": wp,
            "sb": smalls, "ag": ag, "bg": bg,
        })
    return maps


def kernel(x, gn_w, gn_b, qkv_w, qkv_b, proj_w, proj_b):
    import concourse.bass_utils as bu
    nc = build_nc()
    maps = _prep_maps(x, gn_w, gn_b, qkv_w, qkv_b, proj_w, proj_b)
    res = bu.run_bass_kernel_spmd(nc, maps, core_ids=list(range(8)))
    out = np.empty((B, C, HW), np.float32)
    for core in range(8):
        b, half = core // 2, core % 2
        o = res.results[core]["out"]                # [128, 2, NH]
        out[b, :, half * NH:(half + 1) * NH] = \
            o.transpose(1, 0, 2).reshape(C, NH)
    return out.reshape(B, C, 64, 64)



# revision 9
# speedup vs baseline: 1.4138x; 1.4138x over previous
"""AttentionBlock (B=4, C=256, H=W=64) on 8 Trainium2 NeuronCores.

Sharding: data-parallel over (batch, query-half): core i handles batch i//2,
query pixels [half*2048, (half+1)*2048), half = i%2. GroupNorm stats + k/vT
are computed per batch element (duplicated across the pair, cheap); the
O(N^2) attention work is fully sharded 8 ways. No collectives.

v2: all heavy matmuls run in fp8e4 (e4m3) with DoubleRow perf mode: one
matmul contracts K=256 (two 128-partition k-tiles interleaved along a
size-2 free dim) at 0.5 PE cycles per output row -- ~4x the f32r rate.
  1. GroupNorm stats via bn_stats/bn_aggr on the fp8 x copy + tiny f32r
     matmuls with 0/1 group matrices; rstd via ACT Sqrt + DVE reciprocal.
  2. GN fold: qkv conv weights are scaled on device (W' = W .* scale_c,
     cast to fp8); effective channel biases from tiny f32r matmuls on the
     unscaled f32r weights; v-bias pre-broadcast along free dim.
  3. q/k in [c_lo, c_hi, n] fp8 layout (DR-ready); vT[m, c] fp8.
  4. Attention per 512-query chunk, 16 key-block PAIRS (2x128 keys):
     S^T[mb, n] one DR matmul each into a 2-bank PSUM pair tile;
     E = exp(S/16 - 1) as ONE ACT instr per pair ([128,2,512], fp8 out;
     the -1 shift cancels in softmax and keeps E < 100 << fp8e4 max 240);
     O[c, n] += vT-pair^T E (DR, PSUM accum); R[n] += ones^T E (DR M=1).
     proj (fp8 DR) runs on O directly (linear, /R commutes); Rinv = DVE
     reciprocal, partition-broadcast by a K=1 f32r matmul;
     out = proj(O)*Rinv + xh' (xh' = x_half + folded biases, fp32).
All PSUM epilogues run on DVE to keep ACT free for the exp stream.
"""

import numpy as np

B, C, HW = 4, 256, 4096
NH = 2048            # query pixels per core
G, CPG = 32, 8       # groups, channels per group
EPS = 1e-5
MB = HW // 128       # 32 key blocks
NP = MB // 2         # 16 key-block pairs

_cache = {}


def build_nc():
    """Build (and cache) the Bass module."""
    if "nc" in _cache:
        return _cache["nc"]
    import concourse.tile as tile
    from concourse import bacc, mybir

    f32 = mybir.dt.float32
    f32r = mybir.dt.float32r
    fp8 = mybir.dt.float8e4
    AF = mybir.ActivationFunctionType
    OP = mybir.AluOpType
    DR = mybir.MatmulPerfMode.DoubleRow

    nc = bacc.Bacc("TRN2", target_bir_lowering=False, debug=False,
                   enable_asserts=False, num_devices=8)

    # ---- DRAM I/O (host preps everything into device layout) ----
    d_x8 = nc.dram_tensor("x8", [128, 2, HW], fp8, kind="ExternalInput")
    d_x8h = nc.dram_tensor("x8h", [128, 2, NH], fp8, kind="ExternalInput")
    d_xh = nc.dram_tensor("xh", [128, 2, NH], f32, kind="ExternalInput")
    d_wq = nc.dram_tensor("wq", [128, 2, C], f32r, kind="ExternalInput")
    d_wk = nc.dram_tensor("wk", [128, 2, C], f32r, kind="ExternalInput")
    d_wv = nc.dram_tensor("wv", [128, 2, C], f32r, kind="ExternalInput")
    d_wp8 = nc.dram_tensor("wp8", [128, 2, C], fp8, kind="ExternalInput")
    d_sb = nc.dram_tensor("sb", [128, 2, 5], f32, kind="ExternalInput")
    d_ag = nc.dram_tensor("ag", [128, 2, G], f32, kind="ExternalInput")
    d_bg = nc.dram_tensor("bg", [G, 2, 128], f32, kind="ExternalInput")
    d_out = nc.dram_tensor("out", [128, 2, NH], f32, kind="ExternalOutput")

    with tile.TileContext(nc) as tc:
        with (
            tc.tile_pool(name="big", bufs=1) as big,
            tc.tile_pool(name="cst", bufs=1) as cst,
            tc.tile_pool(name="wrk", bufs=2) as wrk,
            tc.tile_pool(name="epool", bufs=4) as epool,
            tc.tile_pool(name="gnp", bufs=1) as gnp,
            tc.tile_pool(name="ps_s", bufs=2, space="PSUM") as ps_s,
            tc.tile_pool(name="ps_o", bufs=1, space="PSUM") as ps_o,
            tc.tile_pool(name="ps_t", bufs=1, space="PSUM") as ps_t,
        ):
            # ---- ACT table warm: sqrt first (GN needs it soon); exp is
            # re-warmed after the GN chain, hidden behind the convs.
            warm = cst.tile([1, 2], f32, tag="warm")
            nc.vector.memset(warm, 1.0)
            nc.scalar.activation(out=warm[:, 1:2], in_=warm[:, 1:2],
                                 func=AF.Sqrt)
            smalls = cst.tile([128, 2, 5], f32, tag="smalls")
            nc.scalar.dma_start(out=smalls, in_=d_sb.ap())
            qb = smalls[:, :, 0:1]
            kb = smalls[:, :, 1:2]
            gb = smalls[:, :, 3:4]
            rbias = smalls[:, :, 4:5]
            ag = cst.tile([128, 2, G], f32, tag="ag")
            nc.scalar.dma_start(out=ag, in_=d_ag.ap())
            bg = cst.tile([G, 2, 128], f32, tag="bg")
            nc.scalar.dma_start(out=bg, in_=d_bg.ap())

            # ---- input loads ----
            x8 = big.tile([128, 2, HW], fp8, tag="x8")
            for ci in range(2):
                for j in range(8):
                    sl = slice(j * 512, (j + 1) * 512)
                    eng = nc.sync if (j % 2 == 0) else nc.scalar
                    eng.dma_start(out=x8[:, ci, sl], in_=d_x8.ap()[:, ci, sl])
            x8h = big.tile([128, 2, NH], fp8, tag="x8h")
            for ci in range(2):
                nc.sync.dma_start(out=x8h[:, ci, :], in_=d_x8h.ap()[:, ci, :])
            wall = cst.tile([128, 2, 3 * C], f32r, tag="wall")
            for i, d in enumerate((d_wq, d_wk, d_wv)):
                nc.scalar.dma_start(out=wall[:, :, i * C:(i + 1) * C], in_=d.ap())
            wp8 = cst.tile([128, 2, C], fp8, tag="wp8")
            nc.scalar.dma_start(out=wp8, in_=d_wp8.ap())
            xh = big.tile([128, 2, NH], f32, tag="xh")
            xo = big.tile([128, 2, NH], f32, tag="xo")  # x_half + rbias
            for ci in range(2):
                for j in range(2):
                    sl = slice(j * 1024, (j + 1) * 1024)
                    nc.sync.dma_start(out=xh[:, ci, sl], in_=d_xh.ap()[:, ci, sl])
                    nc.vector.tensor_scalar(
                        out=xo[:, ci, sl], in0=xh[:, ci, sl],
                        scalar1=rbias[:, ci, :], scalar2=None, op0=OP.add)

            onesc = cst.tile([128, 2], f32, tag="onesc")
            nc.vector.memset(onesc, 1.0)
            epst = cst.tile([G, 1], f32, tag="epst")
            nc.vector.memset(epst, EPS)
            # R lhsT (DR): pair-dim stride must be 16-element aligned
            ones21t = cst.tile([128, 2, 16], fp8, tag="ones21")
            nc.vector.memset(ones21t, 1.0)
            ones21 = ones21t[:, :, 0:1]
            negone = cst.tile([128, 1], f32, tag="negone")  # exp shift
            nc.vector.memset(negone, -2.5)
            onesr = cst.tile([1, 128], f32, tag="onesr")
            nc.vector.memset(onesr, 1.0)
            ones_row = cst.tile([1, 128], f32r, tag="ones_row")  # bcast lhsT
            nc.vector.tensor_copy(out=ones_row, in_=onesr)
            # O is cast to fp8 scaled by 1/64 (O can exceed fp8e4 max 240);
            # the x64 is folded back via the R-broadcast (row of 64s).
            r64s = cst.tile([1, 128], f32, tag="r64s")
            nc.vector.memset(r64s, 64.0)
            row64 = cst.tile([1, 128], f32r, tag="row64")
            nc.vector.tensor_copy(out=row64, in_=r64s)
            inv64 = cst.tile([128, 1], f32, tag="inv64")
            nc.vector.memset(inv64, 1.0 / 64.0)

            # ---- GroupNorm stats (on the fp8 x copy; quantization noise in
            # the stats is ~1e-3 relative, far below the error budget) ----
            bstat = gnp.tile([128, 2, 8, 6], f32, tag="bstat")
            for ci in range(2):
                for j in range(8):
                    nc.vector.bn_stats(
                        out=bstat[:, ci, j, :],
                        in_=x8[:, ci, j * 512:(j + 1) * 512])
            stats2 = gnp.tile([128, 2, 2], f32, tag="stats2")  # (mean, E[x^2])
            tmp1 = gnp.tile([128, 1], f32, tag="tmp1")
            for ci in range(2):
                nc.vector.bn_aggr(out=stats2[:, ci, :], in_=bstat[:, ci, :, :])
                nc.vector.tensor_tensor(
                    out=tmp1, in0=stats2[:, ci, 0:1], in1=stats2[:, ci, 0:1],
                    op=OP.mult)
                nc.vector.tensor_tensor(
                    out=stats2[:, ci, 1:2], in0=stats2[:, ci, 1:2], in1=tmp1,
                    op=OP.add)
            # group sums across partitions: [G, 2] = sum_ci ag[ci]^T stats2[ci]
            pg = ps_t.tile([G, 2], f32, tag="t")
            for ci in range(2):
                nc.tensor.matmul(pg, lhsT=ag[:, ci, :], rhs=stats2[:, ci, :],
                                 start=(ci == 0), stop=(ci == 1))
            # ag carries 1/CPG so pg is directly (mean_g, E[x^2]_g)
            pgs = gnp.tile([G, 2], f32, tag="pgs")
            nc.vector.tensor_copy(out=pgs, in_=pg)
            gst = gnp.tile([G, 4], f32, tag="gst")  # mean^2, var, sd, -
            nc.vector.tensor_tensor(out=gst[:, 0:1], in0=pgs[:, 0:1],
                                    in1=pgs[:, 0:1], op=OP.mult)
            nc.vector.tensor_tensor(out=gst[:, 1:2], in0=pgs[:, 1:2],
                                    in1=gst[:, 0:1], op=OP.subtract)
            gfin = gnp.tile([G, 2], f32, tag="gfin")  # (rstd_g, mean_g*rstd_g)
            nc.scalar.activation(out=gst[:, 2:3], in_=gst[:, 1:2],
                                 func=AF.Sqrt, bias=epst)
            nc.vector.reciprocal(out=gfin[:, 0:1], in_=gst[:, 2:3])
            nc.vector.tensor_tensor(out=gfin[:, 1:2], in0=pgs[:, 0:1],
                                    in1=gfin[:, 0:1], op=OP.mult)
            # bg carries gn_w, so pbc = (scale_c, mean_c*scale_c);
            # bias_c = gn_b - mean_c*scale_c
            scbc = gnp.tile([128, 2, 2], f32, tag="scbc")
            for ci in range(2):
                pbc = ps_t.tile([128, 2], f32, tag="t")
                nc.tensor.matmul(pbc, lhsT=bg[:, ci, :], rhs=gfin,
                                 start=True, stop=True)
                nc.vector.tensor_copy(out=scbc[:, ci, 0:1], in_=pbc[:, 0:1])
                nc.vector.tensor_tensor(out=scbc[:, ci, 1:2], in0=gb[:, ci, :],
                                        in1=pbc[:, 1:2], op=OP.subtract)

            # ---- fold GN into conv weights: W' = W .* scale_c, cast fp8.
            # k section first (kt conv gates the attention loop).
            wall8 = cst.tile([128, 2, 3 * C], fp8, tag="wall8")
            for sec in (1, 0, 2):                 # k, q, v
                for ci in range(2):
                    nc.vector.tensor_scalar(
                        out=wall8[:, ci, sec * C:(sec + 1) * C],
                        in0=wall[:, ci, sec * C:(sec + 1) * C],
                        scalar1=scbc[:, ci, 0:1], scalar2=None, op0=OP.mult)
            w8q, w8k, w8v = (wall8[:, :, i * C:(i + 1) * C] for i in range(3))
            wqs, wks, wvs = (wall[:, :, i * C:(i + 1) * C] for i in range(3))

            # effective channel biases (tiny f32r matmuls on unscaled W):
            # bias_c duplicated to 2 cols for an even f32r moving dim
            bcc = cst.tile([128, 2, 2], f32r, tag="bcc")
            for ci in range(2):
                nc.vector.tensor_copy(out=bcc[:, ci, 0:1], in_=scbc[:, ci, 1:2])
                nc.vector.tensor_copy(out=bcc[:, ci, 1:2], in_=scbc[:, ci, 1:2])
            # qb2/kb2 = b + W^T bias_c   (cols of bias2: [q, k])
            bias2 = gnp.tile([128, 2, 2], f32, tag="bias2")
            for wi, wsl in enumerate((wqs, wks)):
                for cb in range(2):
                    pbias = ps_t.tile([128, 2], f32, tag="t")
                    for ci in range(2):
                        nc.tensor.matmul(
                            pbias,
                            lhsT=wsl[:, ci, cb * 128:(cb + 1) * 128],
                            rhs=bcc[:, ci, :], start=(ci == 0), stop=(ci == 1))
                    nc.vector.tensor_tensor(
                        out=bias2[:, cb, wi:wi + 1], in0=pbias[:, 0:1],
                        in1=(qb if wi == 0 else kb)[:, cb, :], op=OP.add)
            # v bias along FREE dim: vb2[1, c_out] = bias_c^T Wv + vb, then
            # partition-broadcast via a K=1 f32r matmul
            pvb = ps_t.tile([1, 512], f32, tag="t")
            for ci in range(2):
                nc.tensor.matmul(pvb[:, 0:C], lhsT=bcc[:, ci, 0:1],
                                 rhs=wvs[:, ci, :], start=(ci == 0),
                                 stop=(ci == 1))
            vb2r = gnp.tile([1, C], f32r, tag="vb2r")
            nc.vector.tensor_copy(out=vb2r, in_=pvb[:, 0:C])
            vb2b = gnp.tile([128, C], f32, tag="vb2b")
            pvbb = ps_t.tile([128, 512], f32, tag="t")
            nc.tensor.matmul(pvbb[:, 0:C], lhsT=ones_row, rhs=vb2r,
                             start=True, stop=True)
            nc.vector.tensor_copy(out=vb2b, in_=pvbb[:, 0:C])

            nc.scalar.activation(out=warm[:, 0:1], in_=warm[:, 0:1],
                                 func=AF.Exp)

            # ---- qkv convs (fp8 DoubleRow; K=256 in one matmul) ----
            # q first: the attention loop needs qt + ALL of kt.
            qt = big.tile([128, 2, 4, 512], fp8, tag="qt")
            for cb in range(2):
                for t in range(2):
                    pq = ps_s.tile([128, 2, 512], f32, tag="s")
                    for i in range(2):
                        j = 2 * t + i
                        nc.tensor.matmul(
                            pq[:, i, :], lhsT=w8q[:, :, cb * 128:(cb + 1) * 128],
                            rhs=x8h[:, :, j * 512:(j + 1) * 512],
                            start=True, stop=True, perf_mode=DR)
                    nc.vector.tensor_scalar(
                        out=qt[:, cb, 2 * t:2 * t + 2, :], in0=pq,
                        scalar1=bias2[:, cb, 0:1], scalar2=None, op0=OP.add)
            kt = big.tile([128, 2, 8, 512], fp8, tag="kt")
            for cb in range(2):
                for t in range(4):
                    pk = ps_s.tile([128, 2, 512], f32, tag="s")
                    for i in range(2):
                        j = 2 * t + i
                        nc.tensor.matmul(
                            pk[:, i, :], lhsT=w8k[:, :, cb * 128:(cb + 1) * 128],
                            rhs=x8[:, :, j * 512:(j + 1) * 512],
                            start=True, stop=True, perf_mode=DR)
                    nc.vector.tensor_scalar(
                        out=kt[:, cb, 2 * t:2 * t + 2, :], in0=pk,
                        scalar1=bias2[:, cb, 1:2], scalar2=None, op0=OP.add)
            # vb2b duplicated into the pair layout for one-op epilogues
            vb22 = gnp.tile([128, 2, C], f32, tag="vb22")
            nc.vector.tensor_copy(out=vb22[:, 0, :], in_=vb2b)
            nc.vector.tensor_copy(out=vb22[:, 1, :], in_=vb2b)
            vT = big.tile([128, MB, C], fp8, tag="vT")
            for p in range(NP):
                pv = ps_s.tile([128, 2, 512], f32, tag="s")
                for i in range(2):
                    nc.tensor.matmul(
                        pv[:, i, 0:C], lhsT=x8[:, :, (2 * p + i) * 128:(2 * p + i + 1) * 128],
                        rhs=wall8[:, :, 2 * C:3 * C],
                        start=True, stop=True, perf_mode=DR)
                nc.vector.tensor_tensor(
                    out=vT[:, 2 * p:2 * p + 2, :], in0=pv[:, :, 0:C],
                    in1=vb22, op=OP.add)

            # ---- attention ----
            for j in range(NH // 512):
                sl = slice(j * 512, (j + 1) * 512)
                po = ps_o.tile([128, 3, 512], f32, tag="o")  # O c0, O c1, R
                for p in range(NP):
                    ps = ps_s.tile([128, 2, 512], f32, tag="s")
                    for i in range(2):
                        nc.tensor.matmul(
                            ps[:, i, :],
                            lhsT=kt[:, :, (2 * p + i) // 4, ((2 * p + i) % 4) * 128:((2 * p + i) % 4 + 1) * 128],
                            rhs=qt[:, :, j, :], start=True, stop=True,
                            perf_mode=DR)
                    et = epool.tile([128, 2, 512], fp8, tag="et")
                    nc.scalar.activation(out=et, in_=ps, func=AF.Exp,
                                         scale=1.0 / 16.0, bias=negone)
                    for cb in range(2):
                        nc.tensor.matmul(
                            po[:, cb, :],
                            lhsT=vT[:, 2 * p:2 * p + 2, cb * 128:(cb + 1) * 128],
                            rhs=et, start=(p == 0), stop=(p == NP - 1),
                            perf_mode=DR, skip_group_check=True)
                    nc.tensor.matmul(
                        po[0:1, 2, :], lhsT=ones21, rhs=et,
                        start=(p == 0), stop=(p == NP - 1),
                        perf_mode=DR, skip_group_check=True)
                # Free po fast: copy R and both O banks out immediately;
                # the slow single-lane reciprocal then runs on the SBUF copy
                # without holding po.
                rsb = wrk.tile([1, 512], f32, tag="rsb")
                nc.vector.tensor_copy(out=rsb, in_=po[0:1, 2, :])
                onorm = wrk.tile([128, 2, 512], fp8, tag="onorm")
                nc.vector.tensor_scalar(
                    out=onorm[:, 0, :], in0=po[:, 0, :], scalar1=inv64,
                    scalar2=None, op0=OP.mult)
                nc.vector.tensor_scalar(
                    out=onorm[:, 1, :], in0=po[:, 1, :], scalar1=inv64,
                    scalar2=None, op0=OP.mult)
                rinv = wrk.tile([1, 512], f32r, tag="rinv")
                with nc.allow_low_precision(reason="f32r is full fp32 storage"):
                    nc.vector.reciprocal(out=rinv, in_=rsb)
                last = (j == NH // 512 - 1)
                if last:
                    # final chunk: keep PE's last matmuls off the slow
                    # reciprocal chain -- proj first (into ps_t + po's freed
                    # O bank), broadcast goes to po's freed R bank.
                    pps = []
                    for cb in range(2):
                        if cb == 0:
                            pp = ps_t.tile([128, 512], f32, tag="t",
                                           name="pp_last")
                        else:
                            pp = po[:, 1, :]
                        nc.tensor.matmul(
                            pp, lhsT=wp8[:, :, cb * 128:(cb + 1) * 128],
                            rhs=onorm, start=True, stop=True, perf_mode=DR,
                            skip_group_check=True)
                        pps.append(pp)
                    nc.tensor.matmul(po[:, 2, :], lhsT=row64, rhs=rinv,
                                     start=True, stop=True,
                                     skip_group_check=True)
                    rb = wrk.tile([128, 512], f32, tag="rb")
                    nc.vector.tensor_copy(out=rb, in_=po[:, 2, :])
                    for cb in range(2):
                        outt = wrk.tile([128, 512], f32, tag="outt")
                        nc.vector.tensor_tensor(out=outt, in0=pps[cb], in1=rb,
                                                op=OP.mult)
                        nc.vector.tensor_tensor(out=outt, in0=outt,
                                                in1=xo[:, cb, sl], op=OP.add)
                        nc.sync.dma_start(out=d_out.ap()[:, cb, sl], in_=outt)
                else:
                    pbx = ps_t.tile([128, 512], f32, tag="t")
                    nc.tensor.matmul(pbx, lhsT=row64, rhs=rinv,
                                     start=True, stop=True)
                    rb = wrk.tile([128, 512], f32, tag="rb")
                    nc.vector.tensor_copy(out=rb, in_=pbx)
                    for cb in range(2):
                        pp = ps_t.tile([128, 512], f32, tag="t")
                        nc.tensor.matmul(
                            pp, lhsT=wp8[:, :, cb * 128:(cb + 1) * 128],
                            rhs=onorm, start=True, stop=True, perf_mode=DR)
                        outt = wrk.tile([128, 512], f32, tag="outt")
                        nc.vector.tensor_tensor(out=outt, in0=pp, in1=rb,
                                                op=OP.mult)
                        nc.vector.tensor_tensor(out=outt, in0=outt,
                                                in1=xo[:, cb, sl], op=OP.add)
                        nc.sync.dma_start(out=d_out.ap()[:, cb, sl], in_=outt)

    nc.compile()
    _cache["nc"] = nc
    return nc


def _prep_maps(x, gn_w, gn_b, qkv_w, qkv_b, proj_w, proj_b):
    """Host-side sharding + layout prep. Returns list of 8 in_maps."""
    import ml_dtypes
    fp8 = ml_dtypes.float8_e4m3
    x = np.asarray(x, np.float32)
    qkv_w = np.asarray(qkv_w, np.float32)
    qkv_b = np.asarray(qkv_b, np.float32)
    proj_w = np.asarray(proj_w, np.float32)
    proj_b = np.asarray(proj_b, np.float32)
    gn_w = np.asarray(gn_w, np.float32)
    gn_b = np.asarray(gn_b, np.float32)

    def chunked(a):  # [256, ...] -> [128, 2, ...]
        return np.ascontiguousarray(a.reshape(2, 128, *a.shape[1:]).transpose(
            1, 0, *range(2, a.ndim + 1)))

    wq = chunked(qkv_w[0:C].T.copy())          # [c_in, c_out] -> [128,2,C]
    wk = chunked(qkv_w[C:2 * C].T.copy())
    wv = chunked(qkv_w[2 * C:3 * C].T.copy())
    wp8 = chunked(proj_w.T.copy()).astype(fp8)
    rbias = proj_w @ qkv_b[2 * C:3 * C] + proj_b   # v-bias fold + proj bias
    smalls = np.stack([qkv_b[0:C], qkv_b[C:2 * C], gn_w, gn_b, rbias], axis=1)
    smalls = chunked(smalls)

    cidx = np.arange(C)
    ag_full = (cidx[:, None] // CPG == np.arange(G)[None, :]).astype(np.float32)
    ag = chunked(ag_full / CPG)                     # [128, 2, G], carries 1/8
    bg_full = ag_full * gn_w[:, None]               # carries gn_w
    bg = np.ascontiguousarray(
        bg_full.reshape(2, 128, G).transpose(2, 0, 1))  # [G, 2, 128]

    maps = []
    for core in range(8):
        b, half = core // 2, core % 2
        xf = x[b].reshape(C, HW)
        xh = xf[:, half * NH:(half + 1) * NH]
        maps.append({
            "x8": chunked(xf).astype(fp8),
            "x8h": chunked(xh).astype(fp8), "xh": chunked(xh),
            "wq": wq, "wk": wk, "wv": wv, "wp8": wp8,
            "sb": smalls, "ag": ag, "bg": bg,
        })
    return maps


def kernel(x, gn_w, gn_b, qkv_w, qkv_b, proj_w, proj_b):
    import concourse.bass_utils as bu
    nc = build_nc()
    maps = _prep_maps(x, gn_w, gn_b, qkv_w, qkv_b, proj_w, proj_b)
    res = bu.run_bass_kernel_spmd(nc, maps, core_ids=list(range(8)))
    out = np.empty((B, C, HW), np.float32)
    for core in range(8):
        b, half = core // 2, core % 2
        o = res.results[core]["out"]                # [128, 2, NH]
        out[b, :, half * NH:(half + 1) * NH] = \
            o.transpose(1, 0, 2).reshape(C, NH)
    return out.reshape(B, C, 64, 64)
